# revision 29
# baseline (speedup 1.0000x reference)
"""Trainium2 Bass kernel for nn_ClrSelfAttention (CLR self-attention block).

Math reformulation (host-side, exact):
  reference:
    z  = proj_H(log(p + eps))                 # proj_H(x) = x - mean(x, -1)
    q  = proj_H(z @ Wq + bq);  k, v likewise
    att = softmax(q k^T / sqrt(dk)); z_out = att @ v
    out = softmax(proj_H(z_out @ Wo + bo))
  Using proj_H(x) = x @ C (C = I - 11^T/n, symmetric):
    q = (log(p+eps) @ C_D @ Wq + bq) @ C_H = y @ (C_D Wq C_H) + (bq @ C_H)
  so the two mean-subtractions fold into the (double-centered) weights.
  The final proj_H is a per-token constant shift, which softmax ignores,
  so it is dropped. bv passes through attention unchanged (softmax rows
  sum to 1) so it folds into bo: bo_eff = bo + (bv @ C_H) @ Wo; bo_eff is
  applied as an extra contraction row of the output projection (a ones
  row in the stationary z).

Sharding: data-parallel over tokens (no collectives). Each of the 8 cores
owns 512 query tokens (core c: batch b=c//4, rows 512*(c%4)...) and
replicates the K/V projections for its batch. Per-core inputs are
pre-sliced on the host; the key order is permuted so that the core's own
query block forms the first 512 keys (attention is invariant to a
consistent permutation of keys).

Engine plan (vs the 520us baseline):
  - all matmuls bf16 (full PE rate), weights converted host-side
  - S matmuls for a head PAIR run concurrently in the two 64-row halves
    of the PE array (row-tiled systolic packing) -> S cost halves
  - softmax exp is split between the Scalar engine (exact spline) and the
    Vector engine (Schraudolph bitcast approx into bf16) so the exp
    stream keeps up with the PE and the PE never idles (HAM stays warm)
  - output projection computes the natural [token, D] layout directly
    (z stationary / Wo moving) -> the 32 PE transposes are gone
  - softmax denominators bounce through DRAM reshaped to [64,16] so the
    exact DVE reciprocal runs at depth 16 (~0.4us vs 2.7us), then a
    DRAM-broadcast spreads 1/den across partitions for the normalize
  - input pieces stream on the Activation-engine DMA ring so they do not
    queue behind the 6MB weight load on the sync ring; Wo loads during
    attention
  - attention is software-pipelined with a 3-key-tile lag between the
    score matmuls and the weighted-value matmuls; phase-D accumulation
    interleaves two token blocks so only the last pair's matmuls wait on
    the final softmax normalize
Measured: 303.6us HW exec across 8 cores (baseline 520us), rel l2 err
1.2e-3 vs the fp32 reference (tolerance 2e-2).
"""

import sys
import os

for _p in ("/opt/trn_rl_repo", "/root/.axon_site/_ro/trn_rl_repo"):
    if os.path.isdir(_p) and _p not in sys.path:
        sys.path.insert(0, _p)

import numpy as np
import ml_dtypes
from contextlib import ExitStack

import concourse.bass as bass
import concourse.tile as tile
from concourse import bacc, mybir

B, T, D, NH, DK = 2, 2048, 1024, 16, 64
H = NH * DK
EPS = 1e-6
NQ = 512          # query tokens per core
NCH = D // 128    # 8 contraction chunks
HT = H // 128     # 8 head pairs
NKT = T // 128    # 16 key tiles
F32 = mybir.dt.float32
BF16 = mybir.dt.bfloat16
I16 = mybir.dt.int16
BF16NP = ml_dtypes.bfloat16

# Schraudolph approx exp on the Vector engine:
#   bf16_bits(exp(s/8)) ~= int16(s * EXP_A + EXP_B)
# EXP_A folds the 1/sqrt(dk) softmax scale; EXP_B tuned for min rms rel err
# (~1.8% rms / 4.2% max on the weight, ~6e-4 on the final output).
EXP_A = float(0.125 * 128.0 / np.log(2.0))
EXP_B = 16249.0
# key-tiles whose exp runs on the Vector engine (rest: exact Scalar exp);
# the Scalar engine also computes the per-pair 1/den = exp(-ln(den))
DVE_KTS = frozenset((1, 3, 5, 7, 9, 11, 13, 15))


DEBUG_DUMP = bool(os.environ.get("KERNEL_DEBUG_DUMP"))


def build_program(with_qk_bias=False):
    """Build the per-core SPMD program. Returns finalized nc."""
    nc = bacc.Bacc("TRN2", target_bir_lowering=False, debug=False, num_devices=8)

    # ---- DRAM I/O (per-core tensors; contents differ per core) ----
    pt_d = nc.dram_tensor("pt", [D, T], BF16, kind="ExternalInput").ap()
    wq_d = nc.dram_tensor("wq", [D, H], BF16, kind="ExternalInput").ap()
    wk_d = nc.dram_tensor("wk", [D, H], BF16, kind="ExternalInput").ap()
    wv_d = nc.dram_tensor("wv", [D, H], BF16, kind="ExternalInput").ap()
    # Wo rows permuted to (pair, half, dk) + bias row block at j=8
    wom_d = nc.dram_tensor("wom", [9 * 128, D], BF16, kind="ExternalInput").ap()
    if with_qk_bias:
        bqk_d = nc.dram_tensor("bqk", [128, 2 * HT], F32, kind="ExternalInput").ap()
    out_d = nc.dram_tensor("out", [NQ, D], F32, kind="ExternalOutput").ap()
    den_dram = nc.dram_tensor("den_scratch", [NH, NQ], F32,
                              kind="ExternalOutput" if DEBUG_DUMP else "Internal").ap()
    rden_dram = nc.dram_tensor("rden_scratch", [NH, NQ], F32).ap()  # internal
    if DEBUG_DUMP:
        dbg_q = nc.dram_tensor("dbg_q", [128, HT, NQ], BF16, kind="ExternalOutput").ap()
        dbg_k = nc.dram_tensor("dbg_k", [128, HT, T], BF16, kind="ExternalOutput").ap()
        dbg_v = nc.dram_tensor("dbg_v", [128, NKT, NH, 65], BF16, kind="ExternalOutput").ap()
        dbg_z = nc.dram_tensor("dbg_z", [128, 9, NQ], BF16, kind="ExternalOutput").ap()

    pt_r = pt_d.rearrange("(c p) n -> p c n", p=128)      # [128, 8, 2048]
    wq_r = wq_d.rearrange("(c p) h -> p c h", p=128)      # [128, 8, 1024]
    wk_r = wk_d.rearrange("(c p) h -> p c h", p=128)
    wv_r = wv_d.rearrange("(c p) h -> p c h", p=128)
    wom_r = wom_d.rearrange("(j p) d -> p j d", p=128)    # [128, 9, 1024]

    AF = mybir.ActivationFunctionType
    OP = mybir.AluOpType

    with tile.TileContext(nc) as tc, ExitStack() as ctx:
        consts = ctx.enter_context(tc.tile_pool(name="consts", bufs=1))
        persist = ctx.enter_context(tc.tile_pool(name="persist", bufs=1))

        eps_b = consts.tile([128, 1], F32)
        nc.vector.memset(eps_b, EPS)
        zero_b = consts.tile([128, 1], F32)
        nc.vector.memset(zero_b, 0.0)

        # persistent weights (bf16) and activations
        wq_sb = persist.tile([128, NCH, H], BF16)
        wk_sb = persist.tile([128, NCH, H], BF16)
        wv_sb = persist.tile([128, NCH, H], BF16)
        wom_sb = persist.tile([128, 9, D], BF16)
        # wq split in halves on the sync ring (Q-proj can start after the
        # first half); wv follows on sync; wk goes on the Activation ring
        # behind the first input piece so K-proj isn't gated by wq+wv
        nc.sync.dma_start(out=wq_sb[:, :, 0:512], in_=wq_r[:, :, 0:512])
        nc.sync.dma_start(out=wq_sb[:, :, 512:1024], in_=wq_r[:, :, 512:1024])
        nc.sync.dma_start(out=wv_sb, in_=wv_r)
        if with_qk_bias:
            bqk_sb = consts.tile([128, 2 * HT], F32)
            nc.sync.dma_start(out=bqk_sb, in_=bqk_d)

        qT = persist.tile([128, HT, NQ], BF16)            # [dk-in-pair, pair, q]
        kT = persist.tile([128, HT, T], BF16)             # [dk-in-pair, pair, key]
        v_sb = persist.tile([128, NKT, NH, 65], BF16)     # [key-in-tile, kt, head, v|1]
        z_r = persist.tile([128, 9, NQ], BF16)            # attn out + ones row (j=8)

        # ones column (col 64) of v for the in-matmul softmax denominator
        nc.vector.memset(v_sb[:, :, :, 64:65], 1.0)
        # z_r bias row block: partition 0 = 1, partitions 1.. = 0
        nc.vector.memset(z_r[:, 8, :], 0.0)
        nc.vector.memset(z_r[0:1, 8, :], 1.0)

        # ---------------- Phase A: log + Q/K/V projections ----------------
        with tc.tile_pool(name="pieces", bufs=2) as ppool, \
             tc.tile_pool(name="ps_q", bufs=2, space="PSUM") as qps, \
             tc.tile_pool(name="ps_k", bufs=2, space="PSUM") as kps, \
             tc.tile_pool(name="ps_v", bufs=2, space="PSUM") as vps:

            for kc in range(4):
                piece = ppool.tile([128, NCH, 512], BF16, tag="pt")
                nc.scalar.dma_start(out=piece, in_=pt_r[:, :, kc * 512:(kc + 1) * 512])
                if kc == 0:
                    nc.scalar.dma_start(out=wk_sb, in_=wk_r)
                nc.scalar.activation(out=piece, in_=piece, func=AF.Ln,
                                     bias=eps_b, scale=1.0)

                if kc == 0:
                    # Q projection from the first 512 (own-block) tokens
                    for ht in range(HT):
                        ps_q = qps.tile([128, 512], F32)
                        for c in range(NCH):
                            nc.tensor.matmul(ps_q, wq_sb[:, c, ht * 128:(ht + 1) * 128],
                                             piece[:, c, :],
                                             start=(c == 0), stop=(c == NCH - 1))
                        if with_qk_bias:
                            nc.vector.tensor_scalar(
                                out=qT[:, ht, :], in0=ps_q,
                                scalar1=bqk_sb[:, ht:ht + 1], scalar2=None, op0=OP.add)
                        else:
                            nc.vector.tensor_copy(out=qT[:, ht, :], in_=ps_q)

                # K projection (W stationary -> k^T layout)
                for ht in range(HT):
                    ps_k = kps.tile([128, 512], F32)
                    for c in range(NCH):
                        nc.tensor.matmul(ps_k, wk_sb[:, c, ht * 128:(ht + 1) * 128],
                                         piece[:, c, :],
                                         start=(c == 0), stop=(c == NCH - 1))
                    if with_qk_bias:
                        nc.vector.tensor_scalar(
                            out=kT[:, ht, kc * 512:(kc + 1) * 512], in0=ps_k,
                            scalar1=bqk_sb[:, HT + ht:HT + ht + 1], scalar2=None,
                            op0=OP.add)
                    else:
                        nc.vector.tensor_copy(out=kT[:, ht, kc * 512:(kc + 1) * 512],
                                              in_=ps_k)

                # V projection (y chunk stationary -> natural v layout)
                for tk in range(4):
                    for hh in range(2):
                        ps_v = vps.tile([128, 512], F32)
                        for c in range(NCH):
                            nc.tensor.matmul(ps_v, piece[:, c, tk * 128:(tk + 1) * 128],
                                             wv_sb[:, c, hh * 512:(hh + 1) * 512],
                                             start=(c == 0), stop=(c == NCH - 1))
                        nc.vector.tensor_copy(
                            out=v_sb[:, kc * 4 + tk, hh * 8:(hh + 1) * 8, 0:64],
                            in_=ps_v.rearrange("p (j c) -> p j c", c=64))

        # ---------------- Phase C: attention ----------------
        nc.sync.dma_start(out=wom_sb, in_=wom_r)
        # Per head-pair j: S for both heads runs concurrently in the two
        # 64-row halves of the PE array; exp alternates Scalar/Vector; the
        # z matmuls lag two key-tiles behind so the PE never waits on exp.
        with tc.tile_pool(name="ps_s", bufs=2, space="PSUM") as spool, \
             tc.tile_pool(name="ps_z", bufs=2, space="PSUM") as zpool, \
             tc.tile_pool(name="e_p", bufs=6) as epool, \
             tc.tile_pool(name="den_p", bufs=2) as dpool, \
             tc.tile_pool(name="rr_p", bufs=4) as rpool, \
             tc.tile_pool(name="zo_p", bufs=2) as zopool:

            def normalize(j, z_e, z_o):
                # den rows PSUM->SBUF (Scalar Copy: no act-table cost), bounce
                # through DRAM reshaped to [64,16] so the exact DVE reciprocal
                # runs at depth 16 (~0.4us), then broadcast 1/den back.
                den = dpool.tile([65, 2, NQ], F32)
                nc.scalar.activation(out=den[64:65, 0, :], in_=z_e[64:65, :],
                                     func=AF.Copy, bias=0.0, scale=1.0)
                nc.scalar.activation(out=den[64:65, 1, :], in_=z_o[64:65, :],
                                     func=AF.Copy, bias=0.0, scale=1.0)
                nc.sync.dma_start(out=den_dram[2 * j:2 * j + 2, :],
                                  in_=den[64:65, :, :])
                den_w = dpool.tile([64, 16], F32, tag="dw")
                rw = dpool.tile([64, 16], F32, tag="rw")
                nc.sync.dma_start(
                    out=den_w,
                    in_=den_dram[2 * j:2 * j + 2, :].rearrange(
                        "a (p i) -> (a p) i", p=32))
                nc.vector.reciprocal(out=rw, in_=den_w)
                nc.sync.dma_start(
                    out=rden_dram[2 * j:2 * j + 2, :].rearrange(
                        "a (p i) -> (a p) i", p=32),
                    in_=rw)
                rr_e = rpool.tile([64, NQ], F32, tag="rr_e")
                rr_o = rpool.tile([64, NQ], F32, tag="rr_o")
                nc.sync.dma_start(
                    out=rr_e,
                    in_=rden_dram[2 * j:2 * j + 1, :].to_broadcast((64, NQ)))
                nc.sync.dma_start(
                    out=rr_o,
                    in_=rden_dram[2 * j + 1:2 * j + 2, :].to_broadcast((64, NQ)))
                nc.vector.tensor_mul(z_r[0:64, j, :], z_e[0:64, :], rr_e)
                zo_st = zopool.tile([64, NQ], BF16)
                nc.vector.tensor_mul(zo_st, z_o[0:64, :], rr_o)
                nc.sync.dma_start(out=z_r[64:128, j, :], in_=zo_st)

            LAG = 3
            pend = []   # (j, kt, e, z_e, z_o)

            def drain_one():
                j, kt, e, z_e, z_o = pend.pop(0)
                nc.tensor.matmul(z_e, v_sb[:, kt, 2 * j, :], e[:, 0, :],
                                 start=(kt == 0), stop=(kt == NKT - 1))
                nc.tensor.matmul(z_o, v_sb[:, kt, 2 * j + 1, :], e[:, 1, :],
                                 start=(kt == 0), stop=(kt == NKT - 1))
                if kt == NKT - 1:
                    normalize(j, z_e, z_o)

            z_e = z_o = None
            for j in range(HT):
                z_e = zpool.tile([65, NQ], F32, tag="z_e")
                z_o = zpool.tile([65, NQ], F32, tag="z_o")
                for kt in range(NKT):
                    s_ps = spool.tile([128, 2, NQ], F32)
                    # two heads of the pair -> PE rows 0-63 / 64-127, concurrent
                    nc.tensor.matmul(s_ps[:, 0, :],
                                     kT[0:64, j, kt * 128:(kt + 1) * 128],
                                     qT[0:64, j, :], start=True, stop=True)
                    nc.tensor.matmul(s_ps[:, 1, :],
                                     kT[64:128, j, kt * 128:(kt + 1) * 128],
                                     qT[64:128, j, :], start=True, stop=True)
                    e = epool.tile([128, 2, NQ], BF16)
                    if kt in DVE_KTS:
                        nc.vector.tensor_scalar(
                            out=e.bitcast(I16), in0=s_ps,
                            scalar1=EXP_A, scalar2=EXP_B,
                            op0=OP.mult, op1=OP.add)
                    else:
                        nc.scalar.activation(out=e, in_=s_ps, func=AF.Exp,
                                             bias=zero_b, scale=0.125)
                    pend.append((j, kt, e, z_e, z_o))
                    if len(pend) > LAG:
                        drain_one()
            while pend:
                drain_one()

            # ---------- Phase D (inside the C scope: reuses the s_ps PSUM
            # slots so no pool-release barrier separates the last attention
            # matmuls from the output projection; PE stays warm) ----------
            with tc.tile_pool(name="nat_p", bufs=2) as npool, \
                 tc.tile_pool(name="sc_p", bufs=4) as scpool:

                def emit_qb_pair(qbs):
                    tiles = {}
                    for qb in qbs:
                        tiles[qb] = spool.tile([128, 2, 512], F32, name="s_ps")
                    for j in [8] + list(range(7)):
                        for qb in qbs:
                            for dh in range(2):
                                nc.tensor.matmul(tiles[qb][:, dh, :],
                                                 z_r[:, j, qb * 128:(qb + 1) * 128],
                                                 wom_sb[:, j, dh * 512:(dh + 1) * 512],
                                                 start=(j == 8), stop=False)
                    for qb in qbs:
                        for dh in range(2):
                            nc.tensor.matmul(tiles[qb][:, dh, :],
                                             z_r[:, 7, qb * 128:(qb + 1) * 128],
                                             wom_sb[:, 7, dh * 512:(dh + 1) * 512],
                                             start=False, stop=True)
                    for qb in qbs:
                        e_nat = npool.tile([128, 2, 512], F32)
                        sm = scpool.tile([128, 1], F32, tag="sm")
                        nc.scalar.activation(out=e_nat, in_=tiles[qb], func=AF.Exp,
                                             bias=zero_b, scale=1.0, accum_out=sm)
                        rs = scpool.tile([128, 1], F32, tag="rs")
                        nc.vector.reciprocal(out=rs, in_=sm)
                        nc.vector.tensor_scalar_mul(e_nat, e_nat, rs)
                        eng = nc.scalar if qb % 2 == 0 else nc.sync
                        eng.dma_start(out=out_d[qb * 128:(qb + 1) * 128, :],
                                      in_=e_nat.rearrange("p a b -> p (a b)"))

                emit_qb_pair([0, 1])
                emit_qb_pair([2, 3])

        if DEBUG_DUMP:
            nc.sync.dma_start(out=dbg_q, in_=qT)
            nc.sync.dma_start(out=dbg_k, in_=kT)
            nc.sync.dma_start(out=dbg_v, in_=v_sb)
            nc.sync.dma_start(out=dbg_z, in_=z_r)

    nc.finalize()
    return nc


_cached_nc = {}
LAST_RESULTS = None


def _get_nc(with_qk_bias):
    key = bool(with_qk_bias)
    if key not in _cached_nc:
        _cached_nc[key] = build_program(with_qk_bias=key)
    return _cached_nc[key]


def kernel(p, Wq, bq, Wk, bk, Wv, bv, Wo, bo):
    from concourse.bass_utils import run_bass_kernel_spmd

    p = np.asarray(p, np.float32)
    Wq = np.asarray(Wq, np.float32); Wk = np.asarray(Wk, np.float32)
    Wv = np.asarray(Wv, np.float32); Wo = np.asarray(Wo, np.float32)
    bq = np.asarray(bq, np.float32); bk = np.asarray(bk, np.float32)
    bv = np.asarray(bv, np.float32); bo = np.asarray(bo, np.float32)

    # fold the CLR projections into the weights (double-centering, exact)
    def dc(W):
        W = W.astype(np.float64)
        W = W - W.mean(axis=0, keepdims=True)
        W = W - W.mean(axis=1, keepdims=True)
        return W

    Wq2, Wk2, Wv2 = dc(Wq), dc(Wk), dc(Wv)
    bq2 = (bq - bq.mean()).astype(np.float32)
    bk2 = (bk - bk.mean()).astype(np.float32)
    bv2 = (bv - bv.mean()).astype(np.float64)
    bo_eff = (bo.astype(np.float64) + bv2 @ Wo.astype(np.float64)).astype(np.float64)

    # Wo rows permuted to the z_r layout: row (j*128 + p) = Wo[(2j + p//64)*64 + p%64]
    # plus the bias row block at j=8 (partition 0 carries bo_eff via the ones row).
    wom = np.zeros((9 * 128, D), np.float64)
    perm_h = np.empty(H, np.int64)
    for j in range(HT):
        for pp in range(128):
            perm_h[j * 128 + pp] = (2 * j + pp // 64) * 64 + (pp % 64)
    wom[:H] = Wo.astype(np.float64)[perm_h]
    wom[H] = bo_eff

    with_qk_bias = bool(np.any(bq2 != 0) or np.any(bk2 != 0))
    nc = _get_nc(with_qk_bias)

    to_bf16 = lambda a: np.ascontiguousarray(a.astype(np.float32)).astype(BF16NP)
    wq_b = to_bf16(Wq2); wk_b = to_bf16(Wk2); wv_b = to_bf16(Wv2)
    wom_b = to_bf16(wom)
    if with_qk_bias:
        bqk = np.zeros((128, 2 * HT), np.float32)
        for ht in range(HT):
            for pp in range(128):
                hidx = (2 * ht + pp // 64) * 64 + (pp % 64)
                bqk[pp, ht] = bq2[hidx]
                bqk[pp, HT + ht] = bk2[hidx]

    in_maps = []
    for c in range(8):
        b, qo = c // 4, NQ * (c % 4)
        perm = np.r_[qo:qo + NQ, 0:qo, qo + NQ:T]
        pt_c = to_bf16(np.ascontiguousarray(p[b][perm].T))   # [D, T], q block first
        m = {"pt": pt_c, "wq": wq_b, "wk": wk_b, "wv": wv_b, "wom": wom_b}
        if with_qk_bias:
            m["bqk"] = bqk
        in_maps.append(m)

    res = run_bass_kernel_spmd(nc, in_maps, list(range(8)))
    global LAST_RESULTS
    LAST_RESULTS = res

    out = np.empty((B, T, D), np.float32)
    for c in range(8):
        b, qo = c // 4, NQ * (c % 4)
        out[b, qo:qo + NQ, :] = res.results[c]["out"]
    return out


if __name__ == "__main__":
    # smoke-build
    nc = build_program()
    print("built ok:", len(nc.inst_map), "instructions")


# revision 30
# speedup vs baseline: 1.0096x; 1.0096x over previous
"""Trainium2 Bass kernel for nn_ClrSelfAttention (CLR self-attention block).

Math reformulation (host-side, exact):
  reference:
    z  = proj_H(log(p + eps))                 # proj_H(x) = x - mean(x, -1)
    q  = proj_H(z @ Wq + bq);  k, v likewise
    att = softmax(q k^T / sqrt(dk)); z_out = att @ v
    out = softmax(proj_H(z_out @ Wo + bo))
  Using proj_H(x) = x @ C (C = I - 11^T/n, symmetric):
    q = (log(p+eps) @ C_D @ Wq + bq) @ C_H = y @ (C_D Wq C_H) + (bq @ C_H)
  so the two mean-subtractions fold into the (double-centered) weights.
  The final proj_H is a per-token constant shift, which softmax ignores,
  so it is dropped. bv passes through attention unchanged (softmax rows
  sum to 1) so it folds into bo: bo_eff = bo + (bv @ C_H) @ Wo; bo_eff is
  applied as an extra contraction row of the output projection (a ones
  row in the stationary z).

Sharding: data-parallel over tokens (no collectives). Each of the 8 cores
owns 512 query tokens (core c: batch b=c//4, rows 512*(c%4)...) and
replicates the K/V projections for its batch. Per-core inputs are
pre-sliced on the host; the key order is permuted so that the core's own
query block forms the first 512 keys (attention is invariant to a
consistent permutation of keys).

Engine plan (vs the 520us baseline):
  - all matmuls bf16 (full PE rate), weights converted host-side
  - S matmuls for a head PAIR run concurrently in the two 64-row halves
    of the PE array (row-tiled systolic packing) -> S cost halves
  - softmax exp is split between the Scalar engine (exact spline) and the
    Vector engine (Schraudolph bitcast approx into bf16) so the exp
    stream keeps up with the PE and the PE never idles (HAM stays warm)
  - output projection computes the natural [token, D] layout directly
    (z stationary / Wo moving) -> the 32 PE transposes are gone
  - softmax denominators bounce through DRAM reshaped to [64,16] so the
    exact DVE reciprocal runs at depth 16 (~0.4us vs 2.7us), then a
    DRAM-broadcast spreads 1/den across partitions for the normalize
  - input pieces stream on the Activation-engine DMA ring so they do not
    queue behind the 6MB weight load on the sync ring; Wo loads during
    attention
Measured: 312.6us HW exec across 8 cores (baseline 520us), rel l2 err
1.2e-3 vs the fp32 reference (tolerance 2e-2).
"""

import sys
import os

for _p in ("/opt/trn_rl_repo", "/root/.axon_site/_ro/trn_rl_repo"):
    if os.path.isdir(_p) and _p not in sys.path:
        sys.path.insert(0, _p)

import numpy as np
import ml_dtypes
from contextlib import ExitStack

import concourse.bass as bass
import concourse.tile as tile
from concourse import bacc, mybir

B, T, D, NH, DK = 2, 2048, 1024, 16, 64
H = NH * DK
EPS = 1e-6
NQ = 512          # query tokens per core
NCH = D // 128    # 8 contraction chunks
HT = H // 128     # 8 head pairs
NKT = T // 128    # 16 key tiles
F32 = mybir.dt.float32
BF16 = mybir.dt.bfloat16
I16 = mybir.dt.int16
BF16NP = ml_dtypes.bfloat16

# Schraudolph approx exp on the Vector engine:
#   bf16_bits(exp(s/8)) ~= int16(s * EXP_A + EXP_B)
# EXP_A folds the 1/sqrt(dk) softmax scale; EXP_B tuned for min rms rel err
# (~1.8% rms / 4.2% max on the weight, ~6e-4 on the final output).
EXP_A = float(0.125 * 128.0 / np.log(2.0))
EXP_B = 16249.0
# key-tiles whose exp runs on the Vector engine (rest: exact Scalar exp);
# the Scalar engine also computes the per-pair 1/den = exp(-ln(den))
DVE_KTS = frozenset((1, 3, 5, 7, 9, 11, 13, 15))


DEBUG_DUMP = bool(os.environ.get("KERNEL_DEBUG_DUMP"))


def build_program(with_qk_bias=False):
    """Build the per-core SPMD program. Returns finalized nc."""
    nc = bacc.Bacc("TRN2", target_bir_lowering=False, debug=False, num_devices=8)

    # ---- DRAM I/O (per-core tensors; contents differ per core) ----
    pt_d = nc.dram_tensor("pt", [D, T], BF16, kind="ExternalInput").ap()
    wq_d = nc.dram_tensor("wq", [D, H], BF16, kind="ExternalInput").ap()
    wk_d = nc.dram_tensor("wk", [D, H], BF16, kind="ExternalInput").ap()
    wv_d = nc.dram_tensor("wv", [D, H], BF16, kind="ExternalInput").ap()
    # Wo rows permuted to (pair, half, dk) + bias row block at j=8
    wom_d = nc.dram_tensor("wom", [9 * 128, D], BF16, kind="ExternalInput").ap()
    if with_qk_bias:
        bqk_d = nc.dram_tensor("bqk", [128, 2 * HT], F32, kind="ExternalInput").ap()
    out_d = nc.dram_tensor("out", [NQ, D], F32, kind="ExternalOutput").ap()
    den_dram = nc.dram_tensor("den_scratch", [NH, NQ], F32,
                              kind="ExternalOutput" if DEBUG_DUMP else "Internal").ap()
    rden_dram = nc.dram_tensor("rden_scratch", [NH, NQ], F32).ap()  # internal
    if DEBUG_DUMP:
        dbg_q = nc.dram_tensor("dbg_q", [128, HT, NQ], BF16, kind="ExternalOutput").ap()
        dbg_k = nc.dram_tensor("dbg_k", [128, HT, T], BF16, kind="ExternalOutput").ap()
        dbg_v = nc.dram_tensor("dbg_v", [128, NKT, NH, 65], BF16, kind="ExternalOutput").ap()
        dbg_z = nc.dram_tensor("dbg_z", [128, 9, NQ], BF16, kind="ExternalOutput").ap()

    pt_r = pt_d.rearrange("(c p) n -> p c n", p=128)      # [128, 8, 2048]
    wq_r = wq_d.rearrange("(c p) h -> p c h", p=128)      # [128, 8, 1024]
    wk_r = wk_d.rearrange("(c p) h -> p c h", p=128)
    wv_r = wv_d.rearrange("(c p) h -> p c h", p=128)
    wom_r = wom_d.rearrange("(j p) d -> p j d", p=128)    # [128, 9, 1024]

    AF = mybir.ActivationFunctionType
    OP = mybir.AluOpType

    with tile.TileContext(nc) as tc, ExitStack() as ctx:
        consts = ctx.enter_context(tc.tile_pool(name="consts", bufs=1))
        persist = ctx.enter_context(tc.tile_pool(name="persist", bufs=1))

        eps_b = consts.tile([128, 1], F32)
        nc.vector.memset(eps_b, EPS)
        zero_b = consts.tile([128, 1], F32)
        nc.vector.memset(zero_b, 0.0)

        # persistent weights (bf16) and activations
        wq_sb = persist.tile([128, NCH, H], BF16)
        wk_sb = persist.tile([128, NCH, H], BF16)
        wv_sb = persist.tile([128, NCH, H], BF16)
        wom_sb = persist.tile([128, 9, D], BF16)
        # wq split in halves on the sync ring (Q-proj can start after the
        # first half); wv follows on sync; wk goes on the Activation ring
        # behind the first input piece so K-proj isn't gated by wq+wv
        nc.sync.dma_start(out=wq_sb[:, :, 0:512], in_=wq_r[:, :, 0:512])
        nc.sync.dma_start(out=wq_sb[:, :, 512:1024], in_=wq_r[:, :, 512:1024])
        nc.sync.dma_start(out=wv_sb, in_=wv_r)
        if with_qk_bias:
            bqk_sb = consts.tile([128, 2 * HT], F32)
            nc.sync.dma_start(out=bqk_sb, in_=bqk_d)

        qT = persist.tile([128, HT, NQ], BF16)            # [dk-in-pair, pair, q]
        kT = persist.tile([128, HT, T], BF16)             # [dk-in-pair, pair, key]
        v_sb = persist.tile([128, NKT, NH, 65], BF16)     # [key-in-tile, kt, head, v|1]
        z_r = persist.tile([128, 9, NQ], BF16)            # attn out + ones row (j=8)

        # ones column (col 64) of v for the in-matmul softmax denominator
        nc.vector.memset(v_sb[:, :, :, 64:65], 1.0)
        # z_r bias row block: partition 0 = 1, partitions 1.. = 0
        nc.vector.memset(z_r[:, 8, :], 0.0)
        nc.vector.memset(z_r[0:1, 8, :], 1.0)

        # ---------------- Phase A: log + Q/K/V projections ----------------
        with tc.tile_pool(name="pieces", bufs=2) as ppool, \
             tc.tile_pool(name="ps_q", bufs=2, space="PSUM") as qps, \
             tc.tile_pool(name="ps_k", bufs=2, space="PSUM") as kps, \
             tc.tile_pool(name="ps_v", bufs=2, space="PSUM") as vps:

            for kc in range(4):
                piece = ppool.tile([128, NCH, 512], BF16, tag="pt")
                nc.scalar.dma_start(out=piece, in_=pt_r[:, :, kc * 512:(kc + 1) * 512])
                if kc == 0:
                    nc.scalar.dma_start(out=wk_sb, in_=wk_r)
                nc.scalar.activation(out=piece, in_=piece, func=AF.Ln,
                                     bias=eps_b, scale=1.0)

                if kc == 0:
                    # Q projection from the first 512 (own-block) tokens
                    for ht in range(HT):
                        ps_q = qps.tile([128, 512], F32)
                        for c in range(NCH):
                            nc.tensor.matmul(ps_q, wq_sb[:, c, ht * 128:(ht + 1) * 128],
                                             piece[:, c, :],
                                             start=(c == 0), stop=(c == NCH - 1))
                        if with_qk_bias:
                            nc.vector.tensor_scalar(
                                out=qT[:, ht, :], in0=ps_q,
                                scalar1=bqk_sb[:, ht:ht + 1], scalar2=None, op0=OP.add)
                        else:
                            nc.vector.tensor_copy(out=qT[:, ht, :], in_=ps_q)

                # K projection (W stationary -> k^T layout)
                for ht in range(HT):
                    ps_k = kps.tile([128, 512], F32)
                    for c in range(NCH):
                        nc.tensor.matmul(ps_k, wk_sb[:, c, ht * 128:(ht + 1) * 128],
                                         piece[:, c, :],
                                         start=(c == 0), stop=(c == NCH - 1))
                    if with_qk_bias:
                        nc.vector.tensor_scalar(
                            out=kT[:, ht, kc * 512:(kc + 1) * 512], in0=ps_k,
                            scalar1=bqk_sb[:, HT + ht:HT + ht + 1], scalar2=None,
                            op0=OP.add)
                    else:
                        nc.vector.tensor_copy(out=kT[:, ht, kc * 512:(kc + 1) * 512],
                                              in_=ps_k)

                # V projection (y chunk stationary -> natural v layout)
                for tk in range(4):
                    for hh in range(2):
                        ps_v = vps.tile([128, 512], F32)
                        for c in range(NCH):
                            nc.tensor.matmul(ps_v, piece[:, c, tk * 128:(tk + 1) * 128],
                                             wv_sb[:, c, hh * 512:(hh + 1) * 512],
                                             start=(c == 0), stop=(c == NCH - 1))
                        nc.vector.tensor_copy(
                            out=v_sb[:, kc * 4 + tk, hh * 8:(hh + 1) * 8, 0:64],
                            in_=ps_v.rearrange("p (j c) -> p j c", c=64))

        # ---------------- Phase C: attention ----------------
        nc.sync.dma_start(out=wom_sb, in_=wom_r)
        # Per head-pair j: S for both heads runs concurrently in the two
        # 64-row halves of the PE array; exp alternates Scalar/Vector; the
        # z matmuls lag two key-tiles behind so the PE never waits on exp.
        with tc.tile_pool(name="ps_s", bufs=2, space="PSUM") as spool, \
             tc.tile_pool(name="ps_z", bufs=2, space="PSUM") as zpool, \
             tc.tile_pool(name="e_p", bufs=6) as epool, \
             tc.tile_pool(name="den_p", bufs=3) as dpool, \
             tc.tile_pool(name="rr_p", bufs=6) as rpool, \
             tc.tile_pool(name="zo_p", bufs=3) as zopool:

            def normalize(j, z_e, z_o):
                # den rows PSUM->SBUF (Scalar Copy: no act-table cost), bounce
                # through DRAM reshaped to [64,16] so the exact DVE reciprocal
                # runs at depth 16 (~0.4us), then broadcast 1/den back.
                den = dpool.tile([65, 2, NQ], F32)
                nc.scalar.activation(out=den[64:65, 0, :], in_=z_e[64:65, :],
                                     func=AF.Copy, bias=0.0, scale=1.0)
                nc.scalar.activation(out=den[64:65, 1, :], in_=z_o[64:65, :],
                                     func=AF.Copy, bias=0.0, scale=1.0)
                nc.sync.dma_start(out=den_dram[2 * j:2 * j + 2, :],
                                  in_=den[64:65, :, :])
                den_w = dpool.tile([64, 16], F32, tag="dw")
                rw = dpool.tile([64, 16], F32, tag="rw")
                nc.sync.dma_start(
                    out=den_w,
                    in_=den_dram[2 * j:2 * j + 2, :].rearrange(
                        "a (p i) -> (a p) i", p=32))
                nc.vector.reciprocal(out=rw, in_=den_w)
                nc.sync.dma_start(
                    out=rden_dram[2 * j:2 * j + 2, :].rearrange(
                        "a (p i) -> (a p) i", p=32),
                    in_=rw)
                rr_e = rpool.tile([64, NQ], F32, tag="rr_e")
                rr_o = rpool.tile([64, NQ], F32, tag="rr_o")
                nc.sync.dma_start(
                    out=rr_e,
                    in_=rden_dram[2 * j:2 * j + 1, :].to_broadcast((64, NQ)))
                nc.sync.dma_start(
                    out=rr_o,
                    in_=rden_dram[2 * j + 1:2 * j + 2, :].to_broadcast((64, NQ)))
                nc.vector.tensor_mul(z_r[0:64, j, :], z_e[0:64, :], rr_e)
                zo_st = zopool.tile([64, NQ], BF16)
                nc.vector.tensor_mul(zo_st, z_o[0:64, :], rr_o)
                nc.sync.dma_start(out=z_r[64:128, j, :], in_=zo_st)

            LAG = 3
            pend = []   # (j, kt, e, z_e, z_o)

            def drain_one():
                j, kt, e, z_e, z_o = pend.pop(0)
                nc.tensor.matmul(z_e, v_sb[:, kt, 2 * j, :], e[:, 0, :],
                                 start=(kt == 0), stop=(kt == NKT - 1))
                nc.tensor.matmul(z_o, v_sb[:, kt, 2 * j + 1, :], e[:, 1, :],
                                 start=(kt == 0), stop=(kt == NKT - 1))
                if kt == NKT - 1:
                    normalize(j, z_e, z_o)

            z_e = z_o = None
            for j in range(HT):
                z_e = zpool.tile([65, NQ], F32, tag="z_e")
                z_o = zpool.tile([65, NQ], F32, tag="z_o")
                for kt in range(NKT):
                    s_ps = spool.tile([128, 2, NQ], F32)
                    # two heads of the pair -> PE rows 0-63 / 64-127, concurrent
                    nc.tensor.matmul(s_ps[:, 0, :],
                                     kT[0:64, j, kt * 128:(kt + 1) * 128],
                                     qT[0:64, j, :], start=True, stop=True)
                    nc.tensor.matmul(s_ps[:, 1, :],
                                     kT[64:128, j, kt * 128:(kt + 1) * 128],
                                     qT[64:128, j, :], start=True, stop=True)
                    e = epool.tile([128, 2, NQ], BF16)
                    if kt in DVE_KTS:
                        nc.vector.tensor_scalar(
                            out=e.bitcast(I16), in0=s_ps,
                            scalar1=EXP_A, scalar2=EXP_B,
                            op0=OP.mult, op1=OP.add)
                    else:
                        nc.scalar.activation(out=e, in_=s_ps, func=AF.Exp,
                                             bias=zero_b, scale=0.125)
                    pend.append((j, kt, e, z_e, z_o))
                    if len(pend) > LAG:
                        drain_one()
            while pend:
                drain_one()

        if DEBUG_DUMP:
            nc.sync.dma_start(out=dbg_q, in_=qT)
            nc.sync.dma_start(out=dbg_k, in_=kT)
            nc.sync.dma_start(out=dbg_v, in_=v_sb)
            nc.sync.dma_start(out=dbg_z, in_=z_r)

        # ---------------- Phase D: output projection + softmax ----------------
        # natural layout: z chunk stationary, Wo moving -> o[token, d] directly
        with tc.tile_pool(name="ps_o", bufs=2, space="PSUM") as opool, \
             tc.tile_pool(name="nat_p", bufs=2) as npool, \
             tc.tile_pool(name="sc_p", bufs=4) as scpool:

            # accumulate the bias row and pairs 0-6 for two token blocks
            # before the first pair-7 matmul, so the PE has work while the
            # last pair's softmax normalize finishes (j order: 8,0..6 then 7)
            def emit_qb_pair(qbs):
                tiles = {}
                for qb in qbs:
                    tiles[qb] = opool.tile([128, 2, 512], F32, name=f"ps_o_{qb}", tag="ps_o")
                for j in [8] + list(range(7)):
                    for qb in qbs:
                        for dh in range(2):
                            nc.tensor.matmul(tiles[qb][:, dh, :],
                                             z_r[:, j, qb * 128:(qb + 1) * 128],
                                             wom_sb[:, j, dh * 512:(dh + 1) * 512],
                                             start=(j == 8), stop=False)
                for qb in qbs:
                    for dh in range(2):
                        nc.tensor.matmul(tiles[qb][:, dh, :],
                                         z_r[:, 7, qb * 128:(qb + 1) * 128],
                                         wom_sb[:, 7, dh * 512:(dh + 1) * 512],
                                         start=False, stop=True)
                for qb in qbs:
                    e_nat = npool.tile([128, 2, 512], F32)
                    sm = scpool.tile([128, 1], F32, tag="sm")
                    nc.scalar.activation(out=e_nat, in_=tiles[qb], func=AF.Exp,
                                         bias=zero_b, scale=1.0, accum_out=sm)
                    rs = scpool.tile([128, 1], F32, tag="rs")
                    nc.vector.reciprocal(out=rs, in_=sm)
                    nc.vector.tensor_scalar_mul(e_nat, e_nat, rs)
                    eng = nc.scalar if qb % 2 == 0 else nc.sync
                    eng.dma_start(out=out_d[qb * 128:(qb + 1) * 128, :],
                                  in_=e_nat.rearrange("p a b -> p (a b)"))

            emit_qb_pair([0, 1])
            emit_qb_pair([2, 3])

    nc.finalize()
    return nc


_cached_nc = {}
LAST_RESULTS = None


def _get_nc(with_qk_bias):
    key = bool(with_qk_bias)
    if key not in _cached_nc:
        _cached_nc[key] = build_program(with_qk_bias=key)
    return _cached_nc[key]


def kernel(p, Wq, bq, Wk, bk, Wv, bv, Wo, bo):
    from concourse.bass_utils import run_bass_kernel_spmd

    p = np.asarray(p, np.float32)
    Wq = np.asarray(Wq, np.float32); Wk = np.asarray(Wk, np.float32)
    Wv = np.asarray(Wv, np.float32); Wo = np.asarray(Wo, np.float32)
    bq = np.asarray(bq, np.float32); bk = np.asarray(bk, np.float32)
    bv = np.asarray(bv, np.float32); bo = np.asarray(bo, np.float32)

    # fold the CLR projections into the weights (double-centering, exact)
    def dc(W):
        W = W.astype(np.float64)
        W = W - W.mean(axis=0, keepdims=True)
        W = W - W.mean(axis=1, keepdims=True)
        return W

    Wq2, Wk2, Wv2 = dc(Wq), dc(Wk), dc(Wv)
    bq2 = (bq - bq.mean()).astype(np.float32)
    bk2 = (bk - bk.mean()).astype(np.float32)
    bv2 = (bv - bv.mean()).astype(np.float64)
    bo_eff = (bo.astype(np.float64) + bv2 @ Wo.astype(np.float64)).astype(np.float64)

    # Wo rows permuted to the z_r layout: row (j*128 + p) = Wo[(2j + p//64)*64 + p%64]
    # plus the bias row block at j=8 (partition 0 carries bo_eff via the ones row).
    wom = np.zeros((9 * 128, D), np.float64)
    perm_h = np.empty(H, np.int64)
    for j in range(HT):
        for pp in range(128):
            perm_h[j * 128 + pp] = (2 * j + pp // 64) * 64 + (pp % 64)
    wom[:H] = Wo.astype(np.float64)[perm_h]
    wom[H] = bo_eff

    with_qk_bias = bool(np.any(bq2 != 0) or np.any(bk2 != 0))
    nc = _get_nc(with_qk_bias)

    to_bf16 = lambda a: np.ascontiguousarray(a.astype(np.float32)).astype(BF16NP)
    wq_b = to_bf16(Wq2); wk_b = to_bf16(Wk2); wv_b = to_bf16(Wv2)
    wom_b = to_bf16(wom)
    if with_qk_bias:
        bqk = np.zeros((128, 2 * HT), np.float32)
        for ht in range(HT):
            for pp in range(128):
                hidx = (2 * ht + pp // 64) * 64 + (pp % 64)
                bqk[pp, ht] = bq2[hidx]
                bqk[pp, HT + ht] = bk2[hidx]

    in_maps = []
    for c in range(8):
        b, qo = c // 4, NQ * (c % 4)
        perm = np.r_[qo:qo + NQ, 0:qo, qo + NQ:T]
        pt_c = to_bf16(np.ascontiguousarray(p[b][perm].T))   # [D, T], q block first
        m = {"pt": pt_c, "wq": wq_b, "wk": wk_b, "wv": wv_b, "wom": wom_b}
        if with_qk_bias:
            m["bqk"] = bqk
        in_maps.append(m)

    res = run_bass_kernel_spmd(nc, in_maps, list(range(8)))
    global LAST_RESULTS
    LAST_RESULTS = res

    out = np.empty((B, T, D), np.float32)
    for c in range(8):
        b, qo = c // 4, NQ * (c % 4)
        out[b, qo:qo + NQ, :] = res.results[c]["out"]
    return out


if __name__ == "__main__":
    # smoke-build
    nc = build_program()
    print("built ok:", len(nc.inst_map), "instructions")


# revision 31
# speedup vs baseline: 1.0244x; 1.0147x over previous
"""Trainium2 Bass kernel for nn_ClrSelfAttention (CLR self-attention block).

Math reformulation (host-side, exact):
  reference:
    z  = proj_H(log(p + eps))                 # proj_H(x) = x - mean(x, -1)
    q  = proj_H(z @ Wq + bq);  k, v likewise
    att = softmax(q k^T / sqrt(dk)); z_out = att @ v
    out = softmax(proj_H(z_out @ Wo + bo))
  Using proj_H(x) = x @ C (C = I - 11^T/n, symmetric):
    q = (log(p+eps) @ C_D @ Wq + bq) @ C_H = y @ (C_D Wq C_H) + (bq @ C_H)
  so the two mean-subtractions fold into the (double-centered) weights.
  The final proj_H is a per-token constant shift, which softmax ignores,
  so it is dropped. bv passes through attention unchanged (softmax rows
  sum to 1) so it folds into bo: bo_eff = bo + (bv @ C_H) @ Wo; bo_eff is
  applied as an extra contraction row of the output projection (a ones
  row in the stationary z).

Sharding: data-parallel over tokens (no collectives). Each of the 8 cores
owns 512 query tokens (core c: batch b=c//4, rows 512*(c%4)...) and
replicates the K/V projections for its batch. Per-core inputs are
pre-sliced on the host; the key order is permuted so that the core's own
query block forms the first 512 keys (attention is invariant to a
consistent permutation of keys).

Engine plan (vs the 520us baseline):
  - all matmuls bf16 (full PE rate), weights converted host-side
  - S matmuls for a head PAIR run concurrently in the two 64-row halves
    of the PE array (row-tiled systolic packing) -> S cost halves
  - softmax exp is split between the Scalar engine (exact spline) and the
    Vector engine (Schraudolph bitcast approx into bf16) so the exp
    stream keeps up with the PE and the PE never idles (HAM stays warm)
  - output projection computes the natural [token, D] layout directly
    (z stationary / Wo moving) -> the 32 PE transposes are gone
  - softmax denominators bounce through DRAM reshaped to [64,16] so the
    exact DVE reciprocal runs at depth 16 (~0.4us vs 2.7us), then a
    DRAM-broadcast spreads 1/den across partitions for the normalize
  - input pieces stream on the Activation-engine DMA ring so they do not
    queue behind the 6MB weight load on the sync ring; Wo loads during
    attention
Measured: 312.6us HW exec across 8 cores (baseline 520us), rel l2 err
1.2e-3 vs the fp32 reference (tolerance 2e-2).
"""

import sys
import os

for _p in ("/opt/trn_rl_repo", "/root/.axon_site/_ro/trn_rl_repo"):
    if os.path.isdir(_p) and _p not in sys.path:
        sys.path.insert(0, _p)

import numpy as np
import ml_dtypes
from contextlib import ExitStack

import concourse.bass as bass
import concourse.tile as tile
from concourse import bacc, mybir

B, T, D, NH, DK = 2, 2048, 1024, 16, 64
H = NH * DK
EPS = 1e-6
NQ = 512          # query tokens per core
NCH = D // 128    # 8 contraction chunks
HT = H // 128     # 8 head pairs
NKT = T // 128    # 16 key tiles
F32 = mybir.dt.float32
BF16 = mybir.dt.bfloat16
I16 = mybir.dt.int16
BF16NP = ml_dtypes.bfloat16

# Schraudolph approx exp on the Vector engine:
#   bf16_bits(exp(s/8)) ~= int16(s * EXP_A + EXP_B)
# EXP_A folds the 1/sqrt(dk) softmax scale; EXP_B tuned for min rms rel err
# (~1.8% rms / 4.2% max on the weight, ~6e-4 on the final output).
EXP_A = float(0.125 * 128.0 / np.log(2.0))
EXP_B = 16249.0
# key-tiles whose exp runs on the Vector engine (rest: exact Scalar exp);
# the Scalar engine also computes the per-pair 1/den = exp(-ln(den))
DVE_KTS = frozenset((1, 3, 5, 7, 9, 11, 13, 15))


DEBUG_DUMP = bool(os.environ.get("KERNEL_DEBUG_DUMP"))


def build_program(with_qk_bias=False):
    """Build the per-core SPMD program. Returns finalized nc."""
    nc = bacc.Bacc("TRN2", target_bir_lowering=False, debug=False, num_devices=8)

    # ---- DRAM I/O (per-core tensors; contents differ per core) ----
    pt_d = nc.dram_tensor("pt", [D, T], BF16, kind="ExternalInput").ap()
    wq_d = nc.dram_tensor("wq", [D, H], BF16, kind="ExternalInput").ap()
    wk_d = nc.dram_tensor("wk", [D, H], BF16, kind="ExternalInput").ap()
    wv_d = nc.dram_tensor("wv", [D, H], BF16, kind="ExternalInput").ap()
    # Wo rows permuted to (pair, half, dk) + bias row block at j=8
    wom_d = nc.dram_tensor("wom", [9 * 128, D], BF16, kind="ExternalInput").ap()
    if with_qk_bias:
        bqk_d = nc.dram_tensor("bqk", [128, 2 * HT], F32, kind="ExternalInput").ap()
    out_d = nc.dram_tensor("out", [NQ, D], F32, kind="ExternalOutput").ap()
    den_dram = nc.dram_tensor("den_scratch", [NH, NQ], F32,
                              kind="ExternalOutput" if DEBUG_DUMP else "Internal").ap()
    rden_dram = nc.dram_tensor("rden_scratch", [NH, NQ], F32).ap()  # internal
    if DEBUG_DUMP:
        dbg_q = nc.dram_tensor("dbg_q", [128, HT, NQ], BF16, kind="ExternalOutput").ap()
        dbg_k = nc.dram_tensor("dbg_k", [128, HT, T], BF16, kind="ExternalOutput").ap()
        dbg_v = nc.dram_tensor("dbg_v", [128, NKT, NH, 65], BF16, kind="ExternalOutput").ap()
        dbg_z = nc.dram_tensor("dbg_z", [128, 9, NQ], BF16, kind="ExternalOutput").ap()

    pt_r = pt_d.rearrange("(c p) n -> p c n", p=128)      # [128, 8, 2048]
    wq_r = wq_d.rearrange("(c p) h -> p c h", p=128)      # [128, 8, 1024]
    wk_r = wk_d.rearrange("(c p) h -> p c h", p=128)
    wv_r = wv_d.rearrange("(c p) h -> p c h", p=128)
    wom_r = wom_d.rearrange("(j p) d -> p j d", p=128)    # [128, 9, 1024]

    AF = mybir.ActivationFunctionType
    OP = mybir.AluOpType

    with tile.TileContext(nc) as tc, ExitStack() as ctx:
        consts = ctx.enter_context(tc.tile_pool(name="consts", bufs=1))
        persist = ctx.enter_context(tc.tile_pool(name="persist", bufs=1))

        eps_b = consts.tile([128, 1], F32)
        nc.vector.memset(eps_b, EPS)
        zero_b = consts.tile([128, 1], F32)
        nc.vector.memset(zero_b, 0.0)

        # persistent weights (bf16) and activations
        wq_sb = persist.tile([128, NCH, H], BF16)
        wk_sb = persist.tile([128, NCH, H], BF16)
        wv_sb = persist.tile([128, NCH, H], BF16)
        wom_sb = persist.tile([128, 9, D], BF16)
        # wq split in halves on the sync ring (Q-proj can start after the
        # first half); wv follows on sync; wk goes on the Activation ring
        # behind the first input piece so K-proj isn't gated by wq+wv
        nc.sync.dma_start(out=wq_sb[:, :, 0:512], in_=wq_r[:, :, 0:512])
        nc.sync.dma_start(out=wq_sb[:, :, 512:1024], in_=wq_r[:, :, 512:1024])
        nc.sync.dma_start(out=wv_sb, in_=wv_r)
        if with_qk_bias:
            bqk_sb = consts.tile([128, 2 * HT], F32)
            nc.sync.dma_start(out=bqk_sb, in_=bqk_d)

        qT = persist.tile([128, HT, NQ], BF16)            # [dk-in-pair, pair, q]
        kT = persist.tile([128, HT, T], BF16)             # [dk-in-pair, pair, key]
        v_sb = persist.tile([128, NKT, NH, 65], BF16)     # [key-in-tile, kt, head, v|1]
        z_r = persist.tile([128, 9, NQ], BF16)            # attn out + ones row (j=8)

        # ones column (col 64) of v for the in-matmul softmax denominator
        nc.vector.memset(v_sb[:, :, :, 64:65], 1.0)
        # z_r bias row block: partition 0 = 1, partitions 1.. = 0
        nc.vector.memset(z_r[:, 8, :], 0.0)
        nc.vector.memset(z_r[0:1, 8, :], 1.0)

        # ---------------- Phase A: log + Q/K/V projections ----------------
        with tc.tile_pool(name="pieces", bufs=2) as ppool, \
             tc.tile_pool(name="ps_q", bufs=2, space="PSUM") as qps, \
             tc.tile_pool(name="ps_k", bufs=2, space="PSUM") as kps, \
             tc.tile_pool(name="ps_v", bufs=2, space="PSUM") as vps:

            for kc in range(4):
                piece = ppool.tile([128, NCH, 512], BF16, tag="pt")
                nc.scalar.dma_start(out=piece, in_=pt_r[:, :, kc * 512:(kc + 1) * 512])
                if kc == 0:
                    nc.scalar.dma_start(out=wk_sb, in_=wk_r)
                nc.scalar.activation(out=piece, in_=piece, func=AF.Ln,
                                     bias=eps_b, scale=1.0)

                if kc == 0:
                    # Q projection from the first 512 (own-block) tokens
                    for ht in range(HT):
                        ps_q = qps.tile([128, 512], F32)
                        for c in range(NCH):
                            nc.tensor.matmul(ps_q, wq_sb[:, c, ht * 128:(ht + 1) * 128],
                                             piece[:, c, :],
                                             start=(c == 0), stop=(c == NCH - 1))
                        if with_qk_bias:
                            nc.vector.tensor_scalar(
                                out=qT[:, ht, :], in0=ps_q,
                                scalar1=bqk_sb[:, ht:ht + 1], scalar2=None, op0=OP.add)
                        else:
                            nc.vector.tensor_copy(out=qT[:, ht, :], in_=ps_q)

                # K projection (W stationary -> k^T layout)
                for ht in range(HT):
                    ps_k = kps.tile([128, 512], F32)
                    for c in range(NCH):
                        nc.tensor.matmul(ps_k, wk_sb[:, c, ht * 128:(ht + 1) * 128],
                                         piece[:, c, :],
                                         start=(c == 0), stop=(c == NCH - 1))
                    if with_qk_bias:
                        nc.vector.tensor_scalar(
                            out=kT[:, ht, kc * 512:(kc + 1) * 512], in0=ps_k,
                            scalar1=bqk_sb[:, HT + ht:HT + ht + 1], scalar2=None,
                            op0=OP.add)
                    else:
                        nc.vector.tensor_copy(out=kT[:, ht, kc * 512:(kc + 1) * 512],
                                              in_=ps_k)

                # V projection (y chunk stationary -> natural v layout)
                for tk in range(4):
                    for hh in range(2):
                        ps_v = vps.tile([128, 512], F32)
                        for c in range(NCH):
                            nc.tensor.matmul(ps_v, piece[:, c, tk * 128:(tk + 1) * 128],
                                             wv_sb[:, c, hh * 512:(hh + 1) * 512],
                                             start=(c == 0), stop=(c == NCH - 1))
                        nc.vector.tensor_copy(
                            out=v_sb[:, kc * 4 + tk, hh * 8:(hh + 1) * 8, 0:64],
                            in_=ps_v.rearrange("p (j c) -> p j c", c=64))

        # ---------------- Phase C: attention ----------------
        nc.sync.dma_start(out=wom_sb, in_=wom_r)
        # Per head-pair j: S for both heads runs concurrently in the two
        # 64-row halves of the PE array; exp alternates Scalar/Vector; the
        # z matmuls lag two key-tiles behind so the PE never waits on exp.
        with tc.tile_pool(name="ps_s", bufs=2, space="PSUM") as spool, \
             tc.tile_pool(name="ps_z", bufs=2, space="PSUM") as zpool, \
             tc.tile_pool(name="e_p", bufs=6) as epool, \
             tc.tile_pool(name="den_p", bufs=2) as dpool, \
             tc.tile_pool(name="rr_p", bufs=4) as rpool, \
             tc.tile_pool(name="zo_p", bufs=2) as zopool:

            def normalize(j, z_e, z_o):
                # den rows PSUM->SBUF (Scalar Copy: no act-table cost), bounce
                # through DRAM reshaped to [64,16] so the exact DVE reciprocal
                # runs at depth 16 (~0.4us), then broadcast 1/den back.
                den = dpool.tile([65, 2, NQ], F32)
                nc.scalar.activation(out=den[64:65, 0, :], in_=z_e[64:65, :],
                                     func=AF.Copy, bias=0.0, scale=1.0)
                nc.scalar.activation(out=den[64:65, 1, :], in_=z_o[64:65, :],
                                     func=AF.Copy, bias=0.0, scale=1.0)
                nc.sync.dma_start(out=den_dram[2 * j:2 * j + 2, :],
                                  in_=den[64:65, :, :])
                den_w = dpool.tile([64, 16], F32, tag="dw")
                rw = dpool.tile([64, 16], F32, tag="rw")
                nc.sync.dma_start(
                    out=den_w,
                    in_=den_dram[2 * j:2 * j + 2, :].rearrange(
                        "a (p i) -> (a p) i", p=32))
                nc.vector.reciprocal(out=rw, in_=den_w)
                nc.sync.dma_start(
                    out=rden_dram[2 * j:2 * j + 2, :].rearrange(
                        "a (p i) -> (a p) i", p=32),
                    in_=rw)
                rr_e = rpool.tile([64, NQ], F32, tag="rr_e")
                rr_o = rpool.tile([64, NQ], F32, tag="rr_o")
                nc.sync.dma_start(
                    out=rr_e,
                    in_=rden_dram[2 * j:2 * j + 1, :].to_broadcast((64, NQ)))
                nc.sync.dma_start(
                    out=rr_o,
                    in_=rden_dram[2 * j + 1:2 * j + 2, :].to_broadcast((64, NQ)))
                nc.vector.tensor_mul(z_r[0:64, j, :], z_e[0:64, :], rr_e)
                zo_st = zopool.tile([64, NQ], BF16)
                nc.vector.tensor_mul(zo_st, z_o[0:64, :], rr_o)
                nc.sync.dma_start(out=z_r[64:128, j, :], in_=zo_st)

            LAG = 3
            pend = []   # (j, kt, e, z_e, z_o)

            def drain_one():
                j, kt, e, z_e, z_o = pend.pop(0)
                nc.tensor.matmul(z_e, v_sb[:, kt, 2 * j, :], e[:, 0, :],
                                 start=(kt == 0), stop=(kt == NKT - 1))
                nc.tensor.matmul(z_o, v_sb[:, kt, 2 * j + 1, :], e[:, 1, :],
                                 start=(kt == 0), stop=(kt == NKT - 1))
                if kt == NKT - 1:
                    normalize(j, z_e, z_o)

            z_e = z_o = None
            for j in range(HT):
                z_e = zpool.tile([65, NQ], F32, tag="z_e")
                z_o = zpool.tile([65, NQ], F32, tag="z_o")
                for kt in range(NKT):
                    s_ps = spool.tile([128, 2, NQ], F32)
                    # two heads of the pair -> PE rows 0-63 / 64-127, concurrent
                    nc.tensor.matmul(s_ps[:, 0, :],
                                     kT[0:64, j, kt * 128:(kt + 1) * 128],
                                     qT[0:64, j, :], start=True, stop=True)
                    nc.tensor.matmul(s_ps[:, 1, :],
                                     kT[64:128, j, kt * 128:(kt + 1) * 128],
                                     qT[64:128, j, :], start=True, stop=True)
                    e = epool.tile([128, 2, NQ], BF16)
                    if kt in DVE_KTS:
                        nc.vector.tensor_scalar(
                            out=e.bitcast(I16), in0=s_ps,
                            scalar1=EXP_A, scalar2=EXP_B,
                            op0=OP.mult, op1=OP.add)
                    else:
                        nc.scalar.activation(out=e, in_=s_ps, func=AF.Exp,
                                             bias=zero_b, scale=0.125)
                    pend.append((j, kt, e, z_e, z_o))
                    if len(pend) > LAG:
                        drain_one()
            while pend:
                drain_one()

        if DEBUG_DUMP:
            nc.sync.dma_start(out=dbg_q, in_=qT)
            nc.sync.dma_start(out=dbg_k, in_=kT)
            nc.sync.dma_start(out=dbg_v, in_=v_sb)
            nc.sync.dma_start(out=dbg_z, in_=z_r)

        # ---------------- Phase D: output projection + softmax ----------------
        # natural layout: z chunk stationary, Wo moving -> o[token, d] directly
        with tc.tile_pool(name="ps_o", bufs=2, space="PSUM") as opool, \
             tc.tile_pool(name="nat_p", bufs=2) as npool, \
             tc.tile_pool(name="sc_p", bufs=4) as scpool:

            # accumulate the bias row and pairs 0-6 for two token blocks
            # before the first pair-7 matmul, so the PE has work while the
            # last pair's softmax normalize finishes (j order: 8,0..6 then 7)
            def emit_qb_pair(qbs):
                tiles = {}
                for qb in qbs:
                    tiles[qb] = opool.tile([128, 2, 512], F32, name=f"ps_o_{qb}", tag="ps_o")
                for j in [8] + list(range(7)):
                    for qb in qbs:
                        for dh in range(2):
                            nc.tensor.matmul(tiles[qb][:, dh, :],
                                             z_r[:, j, qb * 128:(qb + 1) * 128],
                                             wom_sb[:, j, dh * 512:(dh + 1) * 512],
                                             start=(j == 8), stop=False)
                for qb in qbs:
                    for dh in range(2):
                        nc.tensor.matmul(tiles[qb][:, dh, :],
                                         z_r[:, 7, qb * 128:(qb + 1) * 128],
                                         wom_sb[:, 7, dh * 512:(dh + 1) * 512],
                                         start=False, stop=True)
                for qb in qbs:
                    e_nat = npool.tile([128, 2, 512], F32)
                    sm = scpool.tile([128, 1], F32, tag="sm")
                    nc.scalar.activation(out=e_nat, in_=tiles[qb], func=AF.Exp,
                                         bias=zero_b, scale=1.0, accum_out=sm)
                    rs = scpool.tile([128, 1], F32, tag="rs")
                    nc.vector.reciprocal(out=rs, in_=sm)
                    nc.vector.tensor_scalar_mul(e_nat, e_nat, rs)
                    eng = nc.scalar if qb % 2 == 0 else nc.sync
                    eng.dma_start(out=out_d[qb * 128:(qb + 1) * 128, :],
                                  in_=e_nat.rearrange("p a b -> p (a b)"))

            emit_qb_pair([0, 1])
            emit_qb_pair([2, 3])

    nc.finalize()
    return nc


_cached_nc = {}
LAST_RESULTS = None


def _get_nc(with_qk_bias):
    key = bool(with_qk_bias)
    if key not in _cached_nc:
        _cached_nc[key] = build_program(with_qk_bias=key)
    return _cached_nc[key]


def kernel(p, Wq, bq, Wk, bk, Wv, bv, Wo, bo):
    from concourse.bass_utils import run_bass_kernel_spmd

    p = np.asarray(p, np.float32)
    Wq = np.asarray(Wq, np.float32); Wk = np.asarray(Wk, np.float32)
    Wv = np.asarray(Wv, np.float32); Wo = np.asarray(Wo, np.float32)
    bq = np.asarray(bq, np.float32); bk = np.asarray(bk, np.float32)
    bv = np.asarray(bv, np.float32); bo = np.asarray(bo, np.float32)

    # fold the CLR projections into the weights (double-centering, exact)
    def dc(W):
        W = W.astype(np.float64)
        W = W - W.mean(axis=0, keepdims=True)
        W = W - W.mean(axis=1, keepdims=True)
        return W

    Wq2, Wk2, Wv2 = dc(Wq), dc(Wk), dc(Wv)
    bq2 = (bq - bq.mean()).astype(np.float32)
    bk2 = (bk - bk.mean()).astype(np.float32)
    bv2 = (bv - bv.mean()).astype(np.float64)
    bo_eff = (bo.astype(np.float64) + bv2 @ Wo.astype(np.float64)).astype(np.float64)

    # Wo rows permuted to the z_r layout: row (j*128 + p) = Wo[(2j + p//64)*64 + p%64]
    # plus the bias row block at j=8 (partition 0 carries bo_eff via the ones row).
    wom = np.zeros((9 * 128, D), np.float64)
    perm_h = np.empty(H, np.int64)
    for j in range(HT):
        for pp in range(128):
            perm_h[j * 128 + pp] = (2 * j + pp // 64) * 64 + (pp % 64)
    wom[:H] = Wo.astype(np.float64)[perm_h]
    wom[H] = bo_eff

    with_qk_bias = bool(np.any(bq2 != 0) or np.any(bk2 != 0))
    nc = _get_nc(with_qk_bias)

    to_bf16 = lambda a: np.ascontiguousarray(a.astype(np.float32)).astype(BF16NP)
    wq_b = to_bf16(Wq2); wk_b = to_bf16(Wk2); wv_b = to_bf16(Wv2)
    wom_b = to_bf16(wom)
    if with_qk_bias:
        bqk = np.zeros((128, 2 * HT), np.float32)
        for ht in range(HT):
            for pp in range(128):
                hidx = (2 * ht + pp // 64) * 64 + (pp % 64)
                bqk[pp, ht] = bq2[hidx]
                bqk[pp, HT + ht] = bk2[hidx]

    in_maps = []
    for c in range(8):
        b, qo = c // 4, NQ * (c % 4)
        perm = np.r_[qo:qo + NQ, 0:qo, qo + NQ:T]
        pt_c = to_bf16(np.ascontiguousarray(p[b][perm].T))   # [D, T], q block first
        m = {"pt": pt_c, "wq": wq_b, "wk": wk_b, "wv": wv_b, "wom": wom_b}
        if with_qk_bias:
            m["bqk"] = bqk
        in_maps.append(m)

    res = run_bass_kernel_spmd(nc, in_maps, list(range(8)))
    global LAST_RESULTS
    LAST_RESULTS = res

    out = np.empty((B, T, D), np.float32)
    for c in range(8):
        b, qo = c // 4, NQ * (c % 4)
        out[b, qo:qo + NQ, :] = res.results[c]["out"]
    return out


if __name__ == "__main__":
    # smoke-build
    nc = build_program()
    print("built ok:", len(nc.inst_map), "instructions")
